# revision 25
# baseline (speedup 1.0000x reference)
"""Cross-attention kernel for one TRN2 chip (8 NeuronCores).

Sharding: core = (batch b in {0,1}) x (head-group of 4 heads).  Each core
computes attention for its 4 heads of its batch element and a partial output
projection [N, 1024] in fp16; the host sums the 4 partials per batch (fp32)
and adds the bias.

Host staging: per-core inputs are sliced, cast to bf16 and prepacked into
the exact SBUF layouts (channel-major xT/cT pieces, partition-major
weights) so every load is a few large contiguous descriptors and the device
spends no time on casts/transposes; all FLOPs (projections, scores,
softmax, AV, output projection) run on device.

Device layout per core (matmuls bf16, fp32 PSUM):
  qT/kT  [d'=128 = head-pair, seq 2048] from projection (weights stationary)
  v      [m, 4 heads, 64+1] natural, ones column appended so the AV matmul
         also produces the softmax denominator
  scores sT [m-tile 128, n-half 1024] in PSUM; the two heads of a pair are
  issued as alternating row-group matmuls so they can co-execute on the PE;
  exp on ScalarE only (table preloaded in phase 1; scale folded in; no max
  subtraction -- scores are O(1) here); AV accumulates oT [65, 512] over
  m-tiles.  Normalization: sums rows are DMA-moved to partition 0,
  reciprocal_approx_fast, gpsimd partition_broadcast, DVE multiply.
"""

import ml_dtypes
import numpy as np

import concourse.bass as bass
import concourse.mybir as mybir
import concourse.tile as tile
from concourse import bacc
from concourse.bass import ts
from concourse.bass_utils import run_bass_kernel_spmd

B, N, M, C = 2, 2048, 2048, 1024
HEADS, DH = 16, 64
H_PER = 4                # heads per core
DHC = H_PER * DH         # 256: per-core slice of INNER
SCALE = DH ** -0.5
P = 128
NT = N // P              # 16 n-tiles
MT = M // P              # 16 m-tiles
CCH = C // P             # 8 contraction chunks
FD = 512                 # matmul moving free dim
G4 = 4                   # seq pieces for staged loads
HALF = N // 2            # 1024: attention n-half per pass
N_CORES = 8

F32 = mybir.dt.float32
BF16 = mybir.dt.bfloat16
F16 = mybir.dt.float16
EXP = mybir.ActivationFunctionType.Exp
BF = ml_dtypes.bfloat16

_CACHE = {}


def _build():
    nc = bacc.Bacc("TRN2", target_bir_lowering=False, debug=False,
                   num_devices=N_CORES, num_swdge_queues=4)

    # prepacked: [p, g, cc, n'] with x[n, c] at p=c%128, cc=c//128,
    # g=n//512, n'=n%512
    xT_d = nc.dram_tensor("xT", (P, G4, CCH, FD), BF16,
                          kind="ExternalInput").ap()
    cT_d = nc.dram_tensor("cT", (P, G4, CCH, FD), BF16,
                          kind="ExternalInput").ap()
    msk_d = nc.dram_tensor("msk", (P, MT), F32, kind="ExternalInput").ap()
    wq_d = nc.dram_tensor("wq", (P, CCH, DHC), BF16,
                          kind="ExternalInput").ap()
    wk_d = nc.dram_tensor("wk", (P, CCH, DHC), BF16,
                          kind="ExternalInput").ap()
    wv_d = nc.dram_tensor("wv", (P, CCH, DHC), BF16,
                          kind="ExternalInput").ap()
    wo_d = nc.dram_tensor("wo", (P, 2, C), BF16, kind="ExternalInput").ap()
    y_d = nc.dram_tensor("y", (N, C), F16, kind="ExternalOutput").ap()

    with tile.TileContext(nc) as tc:
        with (
            tc.tile_pool(name="const", bufs=1) as const,
            tc.tile_pool(name="pTp", bufs=4) as pTp,
            tc.tile_pool(name="norm", bufs=2) as norm,
            tc.tile_pool(name="yp", bufs=3) as yp,
        ):
            # ---- persistent SBUF tensors ----
            xT = const.tile([P, G4, CCH, FD], BF16, name="xT")
            cT = const.tile([P, G4, CCH, FD], BF16, name="cT")
            qT = const.tile([P, 2, N], BF16, name="qT")
            kT = const.tile([P, 2, M], BF16, name="kT")
            oTp = const.tile([P, 2, N], BF16, name="oTp")
            v_sb = const.tile([P, MT, H_PER, DH + 1], BF16, name="v_sb")
            wq_sb = const.tile([P, CCH, DHC], BF16, name="wq")
            wk_sb = const.tile([P, CCH, DHC], BF16, name="wk")
            wv_sb = const.tile([P, CCH, DHC], BF16, name="wv")
            wo_sb = const.tile([P, 2, C], BF16, name="wo")
            msk_sb = const.tile([P, MT], F32, name="msk")

            # ---- weights + mask (already packed on host); wk + the first
            # ctx piece come first so the K projection starts ASAP ----
            nc.sync.dma_start(out=wk_sb, in_=wk_d)
            nc.sync.dma_start(out=cT[:, 0], in_=cT_d[:, 0])
            nc.sync.dma_start(out=wv_sb, in_=wv_d)
            nc.sync.dma_start(out=msk_sb, in_=msk_d)
            nc.sync.dma_start(out=wq_sb, in_=wq_d)
            nc.sync.dma_start(out=wo_sb, in_=wo_d)

            # preload the exp table set while the ACT engine is idle so the
            # first real exp doesn't stall the attention pipeline
            warm = const.tile([1, 32], F32, name="warm")
            nc.vector.memset(warm, 0.0)
            nc.scalar.activation(warm, warm, EXP)
            ones1 = const.tile([1, DH], BF16, name="ones1")
            nc.vector.memset(ones1, 1.0)

            # single set of PSUM pools for the whole kernel: projections
            # borrow the attention pools' slots (same tags) so there is no
            # pool-boundary barrier -- the PE never idles long enough at the
            # phase change for the clock gate to re-throttle
            ps_sT_cm = tc.tile_pool(name="ps_sT", bufs=1, space="PSUM")
            ps_sT = ps_sT_cm.__enter__()
            ps_oT_cm = tc.tile_pool(name="ps_oT", bufs=1, space="PSUM")
            ps_oT = ps_oT_cm.__enter__()
            kq_n = [0]

            def proj_T(w_sb, src_T, dst_T, dc, j):
                ps = ps_oT.tile([P, FD], F32, name="kq",
                                tag=f"oT0{kq_n[0] % 2}")
                kq_n[0] += 1
                for cc in range(CCH):
                    nc.tensor.matmul(
                        ps, lhsT=w_sb[:, cc, ts(dc, P)],
                        rhs=src_T[:, j, cc, :],
                        start=(cc == 0), stop=(cc == CCH - 1))
                if (dc + j) % 2 == 0:
                    nc.vector.tensor_copy(dst_T[:, dc, ts(j, FD)], ps)
                else:
                    nc.scalar.copy(dst_T[:, dc, ts(j, FD)], ps)

            # ctx pipeline: load seq-pieces of cT, then the K-proj chunk
            # they enable and the V projection for those m-tiles
            for g in range(G4):
                if g > 0:
                    nc.sync.dma_start(out=cT[:, g], in_=cT_d[:, g])
                for dc in range(2):
                    proj_T(wk_sb, cT, kT, dc, g)
                for mm in range(4):
                    m = 4 * g + mm
                    vp = ps_oT.tile([P, DHC], F32, name="vp",
                                    tag=f"oT1{mm % 2}")
                    for cc in range(CCH):
                        nc.tensor.matmul(
                            vp, lhsT=cT[:, g, cc, ts(mm, P)],
                            rhs=wv_sb[:, cc, :],
                            start=(cc == 0), stop=(cc == CCH - 1))
                    nc.vector.memset(v_sb[:, m], 1.0)
                    nc.vector.tensor_copy(
                        v_sb[:, m, :, 0:DH],
                        vp.rearrange("p (h d) -> p h d", h=H_PER))
                    nc.vector.tensor_scalar_mul(
                        v_sb[:, m], v_sb[:, m], msk_sb[:, m:m + 1])

            # x pipeline + Q-proj chunks (first n-half first)
            for g in range(G4):
                nc.sync.dma_start(out=xT[:, g], in_=xT_d[:, g])
                for dc in range(2):
                    proj_T(wq_sb, xT, qT, dc, g)

            # ---- attention ----
            for pf in range(2):
                for dc in range(2):
                    oT = {}
                    for s in range(2):
                        for jj in range(2):
                            oT[(s, jj)] = ps_oT.tile(
                                [DH + 1, FD], F32, name=f"oT{s}{jj}")
                    for m in range(MT):
                        sT = [ps_sT.tile([P, HALF], F32, name=f"sT{s}")
                              for s in range(2)]
                        # s-major so the first exp can start after 2 matmuls
                        for s in range(2):
                            r = slice(s * DH, (s + 1) * DH)
                            for jj in range(2):
                                nc.tensor.matmul(
                                    sT[s][:, ts(jj, FD)],
                                    lhsT=kT[r, dc, ts(m, P)],
                                    rhs=qT[r, dc,
                                           pf * HALF + jj * FD:
                                           pf * HALF + (jj + 1) * FD],
                                    start=True, stop=True)
                        # the very first tile splits its exps so the AV can
                        # start sooner, shrinking the pipeline-fill bubble
                        # that could otherwise re-throttle the PE clock
                        split = (pf == 0 and dc == 0 and m == 0)
                        for s in range(2):
                            pT = pTp.tile([P, HALF], BF16, name=f"pT{s}")
                            if split:
                                for jj in range(2):
                                    nc.scalar.activation(
                                        pT[:, ts(jj, FD)],
                                        sT[s][:, ts(jj, FD)],
                                        EXP, scale=SCALE)
                                    nc.tensor.matmul(
                                        oT[(s, jj)],
                                        lhsT=v_sb[:, m, 2 * dc + s, :],
                                        rhs=pT[:, ts(jj, FD)],
                                        start=True, stop=False)
                            else:
                                nc.scalar.activation(
                                    pT, sT[s], EXP, scale=SCALE)
                                for jj in range(2):
                                    nc.tensor.matmul(
                                        oT[(s, jj)],
                                        lhsT=v_sb[:, m, 2 * dc + s, :],
                                        rhs=pT[:, ts(jj, FD)],
                                        start=(m == 0), stop=(m == MT - 1))
                    # normalization for this (pf, dc): move each sums row to
                    # partition 0, fast reciprocal, broadcast, multiply
                    o_fs, recs = {}, {}
                    for s in range(2):
                        for jj in range(2):
                            o_f = norm.tile([DH + 1, FD], F32,
                                            name=f"of{s}{jj}")
                            nc.vector.tensor_copy(o_f, oT[(s, jj)])
                            o_fs[(s, jj)] = o_f
                            rec = norm.tile([1, FD], F32, name=f"rec{s}{jj}")
                            nc.sync.dma_start(
                                out=rec, in_=o_f[DH:DH + 1, :])
                            recs[(s, jj)] = rec
                    last = (pf == 1 and dc == 1)
                    for idx, (jj, s) in enumerate(
                            (jj, s) for jj in range(2) for s in range(2)):
                        rec = recs[(s, jj)]
                        nc.vector.reciprocal_approx_fast(rec, rec)
                        if last:
                            # broadcast on the PE (freed oT slots) -- the
                            # gpsimd path is too slow on the critical tail
                            rec_bf = norm.tile([1, FD], BF16,
                                               name=f"recb{s}{jj}")
                            nc.scalar.copy(rec_bf, rec)
                            rb = ps_oT.tile([DH, FD], F32, name="rbp",
                                            tag=f"oT1{idx % 2}")
                            nc.tensor.matmul(
                                rb, lhsT=ones1, rhs=rec_bf,
                                start=True, stop=True)
                        else:
                            rb = norm.tile([DH, FD], F32, name=f"rb{s}{jj}")
                            nc.gpsimd.partition_broadcast(rb, rec)
                        nc.vector.tensor_mul(
                            oTp[s * DH:(s + 1) * DH, dc,
                                pf * HALF + jj * FD:
                                pf * HALF + (jj + 1) * FD],
                            o_fs[(s, jj)][0:DH, :], rb)
            # ---- output projection (fp16 partials, summed on host) ----
            # y PSUM reuses the attention pools' slots (same tags, no pool
            # boundary barrier), half-tiles rotating over 4 slots
            for i in range(NT):
                y_sb = yp.tile([P, C], F16, name="ysb")
                for col in range(2):
                    k = (2 * i + col) % 4
                    pool, tag = ((ps_sT, f"sT{k}") if k < 2
                                 else (ps_oT, f"oT0{k - 2}"))
                    y_ps = pool.tile([P, FD], F32, name="y", tag=tag)
                    for dc in range(2):
                        nc.tensor.matmul(
                            y_ps,
                            lhsT=oTp[:, dc, ts(i, P)],
                            rhs=wo_sb[:, dc, ts(col, FD)],
                            start=(dc == 0), stop=(dc == 1))
                    if col == 0:
                        nc.vector.tensor_copy(y_sb[:, 0:FD], y_ps)
                    else:
                        nc.scalar.copy(y_sb[:, FD:C], y_ps)
                nc.sync.dma_start(out=y_d[ts(i, P), :], in_=y_sb)
            ps_oT_cm.__exit__(None, None, None)
            ps_sT_cm.__exit__(None, None, None)

    nc.compile()
    return nc


def _pack_seq(a):
    # [seq, C] f32 -> [P, G4, CCH, FD] bf16 with a[n, c] at
    # [c % P, n // FD, c // P, n % FD]
    seq = a.shape[0]
    t = a.reshape(G4, FD, CCH, P)          # [g, n', cc, p]
    return np.ascontiguousarray(t.transpose(3, 0, 2, 1)).astype(BF)


def _in_maps(x, context, mask, Wq, Wk, Wv, Wo):
    x = np.asarray(x)
    context = np.asarray(context)
    mask = np.asarray(mask)
    xTh = [_pack_seq(x[b]) for b in range(B)]
    cTh = [_pack_seq(context[b]) for b in range(B)]
    mskh = [np.ascontiguousarray(
        mask[b].astype(np.float32).reshape(MT, P).T) for b in range(B)]
    maps = []
    for core in range(N_CORES):
        b, hg = core // H_PER, core % H_PER
        c0 = hg * DHC
        wq = np.asarray(Wq)[:, c0:c0 + DHC].reshape(CCH, P, DHC)
        wk = np.asarray(Wk)[:, c0:c0 + DHC].reshape(CCH, P, DHC)
        wv = np.asarray(Wv)[:, c0:c0 + DHC].reshape(CCH, P, DHC)
        wo = np.asarray(Wo)[c0:c0 + DHC, :].reshape(2, P, C)
        maps.append({
            "xT": xTh[b],
            "cT": cTh[b],
            "msk": mskh[b],
            "wq": np.ascontiguousarray(wq.transpose(1, 0, 2)).astype(BF),
            "wk": np.ascontiguousarray(wk.transpose(1, 0, 2)).astype(BF),
            "wv": np.ascontiguousarray(wv.transpose(1, 0, 2)).astype(BF),
            "wo": np.ascontiguousarray(wo.transpose(1, 0, 2)).astype(BF),
        })
    return maps


def _gather(results, bo):
    out = np.zeros((B, N, C), dtype=np.float32)
    for core in range(N_CORES):
        out[core // H_PER] += results[core]["y"].astype(np.float32)
    out += np.asarray(bo, dtype=np.float32)
    return out


def kernel(x, context, mask, Wq, Wk, Wv, Wo, bo, **extra_kwargs):
    if "nc" not in _CACHE:
        _CACHE["nc"] = _build()
    nc = _CACHE["nc"]
    maps = _in_maps(x, context, mask, Wq, Wk, Wv, Wo)
    res = run_bass_kernel_spmd(nc, maps, core_ids=list(range(N_CORES)),
                               **extra_kwargs)
    out = _gather(res.results, bo)
    if extra_kwargs:
        _CACHE["last_result"] = res
    return out


# revision 28
# speedup vs baseline: 1.4038x; 1.4038x over previous
"""Cross-attention kernel for one TRN2 chip (8 NeuronCores).

Sharding: core = (batch b in {0,1}) x (head-group of 4 heads).  Each core
computes attention for its 4 heads of its batch element and a partial output
projection [N, 1024] in fp16; the host sums the 4 partials per batch (fp32)
and adds the bias.

Host staging: per-core inputs are sliced, cast to bf16 and prepacked into
the exact SBUF layouts (channel-major xT/cT pieces, partition-major
weights) so every load is a few large contiguous descriptors and the device
spends no time on casts/transposes; all FLOPs (projections, scores,
softmax, AV, output projection) run on device.

Device layout per core (matmuls bf16, fp32 PSUM):
  qT/kT  [d'=128 = head-pair, seq 2048] from projection (weights stationary)
  v      [m, 4 heads, 64+1] natural, ones column appended so the AV matmul
         also produces the softmax denominator
  scores sT [m-tile 128, n-half 1024] in PSUM; the two heads of a pair are
  issued as alternating row-group matmuls so they can co-execute on the PE;
  exp on ScalarE only (table preloaded in phase 1; scale folded in; no max
  subtraction -- scores are O(1) here); AV accumulates oT [65, 512] over
  m-tiles.  Normalization: sums rows are DMA-moved to partition 0,
  reciprocal_approx_fast, gpsimd partition_broadcast, DVE multiply.
"""

import ml_dtypes
import numpy as np

import concourse.bass as bass
import concourse.mybir as mybir
import concourse.tile as tile
from concourse import bacc
from concourse.bass import ts
from concourse.bass_utils import run_bass_kernel_spmd

B, N, M, C = 2, 2048, 2048, 1024
HEADS, DH = 16, 64
H_PER = 4                # heads per core
DHC = H_PER * DH         # 256: per-core slice of INNER
SCALE = DH ** -0.5
P = 128
NT = N // P              # 16 n-tiles
MT = M // P              # 16 m-tiles
CCH = C // P             # 8 contraction chunks
FD = 512                 # matmul moving free dim
G4 = 4                   # seq pieces for staged loads
HALF = N // 2            # 1024: attention n-half per pass
N_CORES = 8

F32 = mybir.dt.float32
BF16 = mybir.dt.bfloat16
F16 = mybir.dt.float16
EXP = mybir.ActivationFunctionType.Exp
BF = ml_dtypes.bfloat16

_CACHE = {}


def _build():
    nc = bacc.Bacc("TRN2", target_bir_lowering=False, debug=False,
                   num_devices=N_CORES, num_swdge_queues=4)

    # prepacked: [p, g, cc, n'] with x[n, c] at p=c%128, cc=c//128,
    # g=n//512, n'=n%512
    xT_d = nc.dram_tensor("xT", (P, G4, CCH, FD), BF16,
                          kind="ExternalInput").ap()
    cT_d = nc.dram_tensor("cT", (P, G4, CCH, FD), BF16,
                          kind="ExternalInput").ap()
    msk_d = nc.dram_tensor("msk", (P, MT), F32, kind="ExternalInput").ap()
    wq_d = nc.dram_tensor("wq", (P, CCH, DHC), BF16,
                          kind="ExternalInput").ap()
    wk_d = nc.dram_tensor("wk", (P, CCH, DHC), BF16,
                          kind="ExternalInput").ap()
    wv_d = nc.dram_tensor("wv", (P, CCH, DHC), BF16,
                          kind="ExternalInput").ap()
    wo_d = nc.dram_tensor("wo", (P, 2, C), BF16, kind="ExternalInput").ap()
    y_d = nc.dram_tensor("y", (N, C), F16, kind="ExternalOutput").ap()

    with tile.TileContext(nc) as tc:
        with (
            tc.tile_pool(name="const", bufs=1) as const,
            tc.tile_pool(name="pTp", bufs=4) as pTp,
            tc.tile_pool(name="norm", bufs=2) as norm,
            tc.tile_pool(name="yp", bufs=3) as yp,
        ):
            # ---- persistent SBUF tensors ----
            xT = const.tile([P, G4, CCH, FD], BF16, name="xT")
            cT = const.tile([P, G4, CCH, FD], BF16, name="cT")
            qT = const.tile([P, 2, N], BF16, name="qT")
            kT = const.tile([P, 2, M], BF16, name="kT")
            oTp = const.tile([P, 2, N], BF16, name="oTp")
            v_sb = const.tile([P, MT, H_PER, DH + 1], BF16, name="v_sb")
            wq_sb = const.tile([P, CCH, DHC], BF16, name="wq")
            wk_sb = const.tile([P, CCH, DHC], BF16, name="wk")
            wv_sb = const.tile([P, CCH, DHC], BF16, name="wv")
            wo_sb = const.tile([P, 2, C], BF16, name="wo")
            msk_sb = const.tile([P, MT], F32, name="msk")

            # ---- weights + mask (already packed on host); wk + the first
            # ctx piece come first so the K projection starts ASAP ----
            nc.sync.dma_start(out=wk_sb, in_=wk_d)
            nc.sync.dma_start(out=cT[:, 0], in_=cT_d[:, 0])
            nc.sync.dma_start(out=wv_sb, in_=wv_d)
            nc.sync.dma_start(out=msk_sb, in_=msk_d)
            nc.sync.dma_start(out=wq_sb, in_=wq_d)
            nc.sync.dma_start(out=wo_sb, in_=wo_d)

            # preload the exp table set while the ACT engine is idle so the
            # first real exp doesn't stall the attention pipeline
            warm = const.tile([1, 32], F32, name="warm")
            nc.vector.memset(warm, 0.0)
            nc.scalar.activation(warm, warm, EXP)
            ones1 = const.tile([1, DH], BF16, name="ones1")
            nc.vector.memset(ones1, 1.0)

            # single set of PSUM pools for the whole kernel: projections
            # borrow the attention pools' slots (same tags) so there is no
            # pool-boundary barrier -- the PE never idles long enough at the
            # phase change for the clock gate to re-throttle
            ps_sT_cm = tc.tile_pool(name="ps_sT", bufs=1, space="PSUM")
            ps_sT = ps_sT_cm.__enter__()
            ps_oT_cm = tc.tile_pool(name="ps_oT", bufs=1, space="PSUM")
            ps_oT = ps_oT_cm.__enter__()
            kq_n = [0]

            def proj_T(w_sb, src_T, dst_T, dc, j, tagbase="oT0"):
                ps = ps_oT.tile([P, FD], F32, name="kq",
                                tag=f"{tagbase}{kq_n[0] % 2}")
                kq_n[0] += 1
                for cc in range(CCH):
                    nc.tensor.matmul(
                        ps, lhsT=w_sb[:, cc, ts(dc, P)],
                        rhs=src_T[:, j, cc, :],
                        start=(cc == 0), stop=(cc == CCH - 1))
                if (dc + j) % 2 == 0:
                    nc.vector.tensor_copy(dst_T[:, dc, ts(j, FD)], ps)
                else:
                    nc.scalar.copy(dst_T[:, dc, ts(j, FD)], ps)

            # ctx pipeline: load seq-pieces of cT, then the K-proj chunk
            # they enable and the V projection for those m-tiles
            for g in range(G4):
                if g > 0:
                    nc.sync.dma_start(out=cT[:, g], in_=cT_d[:, g])
                for dc in range(2):
                    proj_T(wk_sb, cT, kT, dc, g)
                for mm in range(4):
                    m = 4 * g + mm
                    vp = ps_oT.tile([P, DHC], F32, name="vp",
                                    tag=f"oT1{mm % 2}")
                    for cc in range(CCH):
                        nc.tensor.matmul(
                            vp, lhsT=cT[:, g, cc, ts(mm, P)],
                            rhs=wv_sb[:, cc, :],
                            start=(cc == 0), stop=(cc == CCH - 1))
                    nc.vector.memset(v_sb[:, m], 1.0)
                    nc.vector.tensor_copy(
                        v_sb[:, m, :, 0:DH],
                        vp.rearrange("p (h d) -> p h d", h=H_PER))
                    nc.vector.tensor_scalar_mul(
                        v_sb[:, m], v_sb[:, m], msk_sb[:, m:m + 1])

            # x pipeline + Q-proj for the first n-half; the second half's
            # projections are interleaved into the first attention tile so
            # the PE stays dense across the phase boundary (an idle window
            # there can latch the PE clock gate cold for the whole phase)
            for g in range(2):
                nc.sync.dma_start(out=xT[:, g], in_=xT_d[:, g])
                for dc in range(2):
                    proj_T(wq_sb, xT, qT, dc, g)
            for g in range(2, G4):
                nc.sync.dma_start(out=xT[:, g], in_=xT_d[:, g])

            # ---- attention ----
            for pf in range(2):
                for dc in range(2):
                    oT = {}
                    for s in range(2):
                        for jj in range(2):
                            oT[(s, jj)] = ps_oT.tile(
                                [DH + 1, FD], F32, name=f"oT{s}{jj}")
                    for m in range(MT):
                        sT = [ps_sT.tile([P, HALF], F32, name=f"sT{s}")
                              for s in range(2)]
                        # s-major so the first exp can start after 2 matmuls
                        for s in range(2):
                            r = slice(s * DH, (s + 1) * DH)
                            for jj in range(2):
                                nc.tensor.matmul(
                                    sT[s][:, ts(jj, FD)],
                                    lhsT=kT[r, dc, ts(m, P)],
                                    rhs=qT[r, dc,
                                           pf * HALF + jj * FD:
                                           pf * HALF + (jj + 1) * FD],
                                    start=True, stop=True)
                        # the very first tile interleaves the second-half Q
                        # projections between its exps and AVs: the kq slots
                        # are still free until the first AVs land, and the
                        # dense PE stream prevents the clock gate from
                        # latching cold at the phase boundary
                        first = (pf == 0 and dc == 0 and m == 0)
                        for s in range(2):
                            pT = pTp.tile([P, HALF], BF16, name=f"pT{s}")
                            nc.scalar.activation(pT, sT[s], EXP, scale=SCALE)
                            if first:
                                g = 2 + s
                                for dcq in range(2):
                                    proj_T(wq_sb, xT, qT, dcq, g,
                                           tagbase=f"oT{s}")
                            for jj in range(2):
                                nc.tensor.matmul(
                                    oT[(s, jj)],
                                    lhsT=v_sb[:, m, 2 * dc + s, :],
                                    rhs=pT[:, ts(jj, FD)],
                                    start=(m == 0), stop=(m == MT - 1))
                    # normalization for this (pf, dc): move each sums row to
                    # partition 0, fast reciprocal, broadcast, multiply
                    o_fs, recs = {}, {}
                    for s in range(2):
                        for jj in range(2):
                            o_f = norm.tile([DH + 1, FD], F32,
                                            name=f"of{s}{jj}")
                            nc.vector.tensor_copy(o_f, oT[(s, jj)])
                            o_fs[(s, jj)] = o_f
                            rec = norm.tile([1, FD], F32, name=f"rec{s}{jj}")
                            nc.sync.dma_start(
                                out=rec, in_=o_f[DH:DH + 1, :])
                            recs[(s, jj)] = rec
                    last = (pf == 1 and dc == 1)
                    for idx, (jj, s) in enumerate(
                            (jj, s) for jj in range(2) for s in range(2)):
                        rec = recs[(s, jj)]
                        nc.vector.reciprocal_approx_fast(rec, rec)
                        if last:
                            # broadcast on the PE (freed oT slots) -- the
                            # gpsimd path is too slow on the critical tail
                            rec_bf = norm.tile([1, FD], BF16,
                                               name=f"recb{s}{jj}")
                            nc.scalar.copy(rec_bf, rec)
                            rb = ps_oT.tile([DH, FD], F32, name="rbp",
                                            tag=f"oT1{idx % 2}")
                            nc.tensor.matmul(
                                rb, lhsT=ones1, rhs=rec_bf,
                                start=True, stop=True)
                        else:
                            rb = norm.tile([DH, FD], F32, name=f"rb{s}{jj}")
                            nc.gpsimd.partition_broadcast(rb, rec)
                        nc.vector.tensor_mul(
                            oTp[s * DH:(s + 1) * DH, dc,
                                pf * HALF + jj * FD:
                                pf * HALF + (jj + 1) * FD],
                            o_fs[(s, jj)][0:DH, :], rb)
            # ---- output projection (fp16 partials, summed on host) ----
            # y PSUM reuses the attention pools' slots (same tags, no pool
            # boundary barrier), half-tiles rotating over 4 slots
            for i in range(NT):
                y_sb = yp.tile([P, C], F16, name="ysb")
                for col in range(2):
                    k = (2 * i + col) % 4
                    pool, tag = ((ps_sT, f"sT{k}") if k < 2
                                 else (ps_oT, f"oT0{k - 2}"))
                    y_ps = pool.tile([P, FD], F32, name="y", tag=tag)
                    for dc in range(2):
                        nc.tensor.matmul(
                            y_ps,
                            lhsT=oTp[:, dc, ts(i, P)],
                            rhs=wo_sb[:, dc, ts(col, FD)],
                            start=(dc == 0), stop=(dc == 1))
                    if col == 0:
                        nc.vector.tensor_copy(y_sb[:, 0:FD], y_ps)
                    else:
                        nc.scalar.copy(y_sb[:, FD:C], y_ps)
                nc.sync.dma_start(out=y_d[ts(i, P), :], in_=y_sb)
            ps_oT_cm.__exit__(None, None, None)
            ps_sT_cm.__exit__(None, None, None)

    nc.compile()
    return nc


def _pack_seq(a):
    # [seq, C] f32 -> [P, G4, CCH, FD] bf16 with a[n, c] at
    # [c % P, n // FD, c // P, n % FD]
    seq = a.shape[0]
    t = a.reshape(G4, FD, CCH, P)          # [g, n', cc, p]
    return np.ascontiguousarray(t.transpose(3, 0, 2, 1)).astype(BF)


def _in_maps(x, context, mask, Wq, Wk, Wv, Wo):
    x = np.asarray(x)
    context = np.asarray(context)
    mask = np.asarray(mask)
    xTh = [_pack_seq(x[b]) for b in range(B)]
    cTh = [_pack_seq(context[b]) for b in range(B)]
    mskh = [np.ascontiguousarray(
        mask[b].astype(np.float32).reshape(MT, P).T) for b in range(B)]
    maps = []
    for core in range(N_CORES):
        b, hg = core // H_PER, core % H_PER
        c0 = hg * DHC
        wq = np.asarray(Wq)[:, c0:c0 + DHC].reshape(CCH, P, DHC)
        wk = np.asarray(Wk)[:, c0:c0 + DHC].reshape(CCH, P, DHC)
        wv = np.asarray(Wv)[:, c0:c0 + DHC].reshape(CCH, P, DHC)
        wo = np.asarray(Wo)[c0:c0 + DHC, :].reshape(2, P, C)
        maps.append({
            "xT": xTh[b],
            "cT": cTh[b],
            "msk": mskh[b],
            "wq": np.ascontiguousarray(wq.transpose(1, 0, 2)).astype(BF),
            "wk": np.ascontiguousarray(wk.transpose(1, 0, 2)).astype(BF),
            "wv": np.ascontiguousarray(wv.transpose(1, 0, 2)).astype(BF),
            "wo": np.ascontiguousarray(wo.transpose(1, 0, 2)).astype(BF),
        })
    return maps


def _gather(results, bo):
    out = np.zeros((B, N, C), dtype=np.float32)
    for core in range(N_CORES):
        out[core // H_PER] += results[core]["y"].astype(np.float32)
    out += np.asarray(bo, dtype=np.float32)
    return out


def kernel(x, context, mask, Wq, Wk, Wv, Wo, bo, **extra_kwargs):
    if "nc" not in _CACHE:
        _CACHE["nc"] = _build()
    nc = _CACHE["nc"]
    maps = _in_maps(x, context, mask, Wq, Wk, Wv, Wo)
    res = run_bass_kernel_spmd(nc, maps, core_ids=list(range(N_CORES)),
                               **extra_kwargs)
    out = _gather(res.results, bo)
    if extra_kwargs:
        _CACHE["last_result"] = res
    return out
